# revision 33
# baseline (speedup 1.0000x reference)
"""LSTM (B=64, T=512, D=64, U=256) + dense head, Trainium2 Bass kernel.

Sharding: data-parallel over batch. 8 cores x 8 sequences each, no
collectives. Everything on-device lives in "transposed" layout
[feature, batch] so gates sit on partitions and elementwise ops run with
all 128 lanes busy.

The 512-step recurrence is split into two chained 256-step programs
(a single 512-step program exceeds a per-engine instruction-count limit
on hardware); h/c state passes through DRAM between the launches.

Per recurrence step (gate columns host-permuted to [i, f, o, g]):
  PE:  two PSUM tiles in separate banks — sigmoid slots zp[128,6,BL] and
       relu-gate slots zg[128,2,BL]. Each opens with a full-tile zeros
       matmul (start=True clears has_written for the WHOLE bank, so one
       covering matmul must set every element's bit), then per slot: a
       W-matmul (wbᵀ@x_t with a ones-row folding in the bias — no xz
       precompute phase) and two U-matmuls (upᵀ@h_{t-1}, bf16) accumulate.
       Step t+1's W-matmuls are emitted after step t's U-matmuls so they
       run in the PE idle window.
  ACT: one sigmoid over zp -> zs slots [i, f, o]. A post-schedule pass
       (_hoist_sigmoid_waits) moves its PE wait out of the SEQ-blocking
       EventSemaphore so the decode happens in the idle window.
  DVE: rg = max(zg, 0) into cw[0:2] runs DURING the sigmoid (its own
       PSUM bank avoids the single-reader serialization with ACT), and
       cw[2:4] already holds c_{t-1}, so the critical chain is 3 ops:
         t12 = zs[0:4] * cw[0:4]        ([i*rg, f*c] in one op)
         c'  = t12[0:2] + t12[2:4]      (-> cw_next[2:4])
         h   = c' * o                   (c' >= 0 always, relu elided)
       The same-engine completion waits between them (~95ns) are
       hardware-required: the DVE does not forward back-to-back writes.
All pools use bufs>=3 so WAR deps land >=2 steps in the past.
"""

import numpy as np
import ml_dtypes

import concourse.bass as bass
import concourse.bacc as bacc
import concourse.mybir as mybir
import concourse.tile as tile
from concourse.bass_utils import run_bass_kernel_spmd
from concourse.masks import make_identity

B, T, D, NU = 64, 512, 64, 256
G = 4 * NU  # 1024
NCORES = 8
BL = B // NCORES  # batch per core
TSPLIT = 256  # steps per launch
DB = D + 1  # x rows + ones row (bias)

F32 = mybir.dt.float32
BF16 = mybir.dt.bfloat16
AF = mybir.ActivationFunctionType
ALU = mybir.AluOpType

# Original gate packing along the 4U axis is [i, f, g, o] (Keras order).
# On-device slot order is [i, f, o, g]: sigmoid gates contiguous in slots
# 0..5 (i,f first so t12 pairs them with [rg, c]), relu gate (g) in 6..7.
PERM = np.concatenate(
    [
        np.arange(0, 256),  # i
        np.arange(256, 512),  # f
        np.arange(768, 1024),  # o
        np.arange(512, 768),  # g
    ]
)


def build_program(t_steps: int = TSPLIT, chained: bool = True):
    tb = t_steps * BL
    nc = bacc.Bacc()

    xt_d = nc.dram_tensor("xt", [DB, tb], BF16, kind="ExternalInput")
    wb_d = nc.dram_tensor("wb", [DB, G], BF16, kind="ExternalInput")
    up_d = nc.dram_tensor("up", [NU, G], BF16, kind="ExternalInput")
    dw_d = nc.dram_tensor("dw", [NU, 1], BF16, kind="ExternalInput")
    out_d = nc.dram_tensor("out", [tb], F32, kind="ExternalOutput")
    if chained:
        hin_d = nc.dram_tensor("hin", [128, 2 * BL], BF16, kind="ExternalInput")
        cin_d = nc.dram_tensor("cin", [128, 2 * BL], F32, kind="ExternalInput")
        hout_d = nc.dram_tensor("hout", [128, 2 * BL], BF16, kind="ExternalOutput")
        cout_d = nc.dram_tensor("cout", [128, 2 * BL], F32, kind="ExternalOutput")

    cs = min(512, tb)  # free-dim chunk for the dense head
    n_chunks = tb // cs
    spt = cs // BL  # steps per head chunk

    with tile.TileContext(nc) as tc:
        with (
            tc.tile_pool(name="const", bufs=1) as const,
            tc.tile_pool(name="state", bufs=1) as state,
            tc.tile_pool(name="zsp", bufs=6) as zsp,
            tc.tile_pool(name="tmp", bufs=6) as tmp,
            tc.tile_pool(name="cwp", bufs=4) as cwp,
            tc.tile_pool(name="zpsum", bufs=3, space="PSUM") as zpsum,
            tc.tile_pool(name="zgsum", bufs=3, space="PSUM") as zgsum,
            tc.tile_pool(name="ppsum", bufs=2, space="PSUM") as ppsum,
        ):
            xt = const.tile([DB, tb], BF16)
            wb = const.tile([DB, G], BF16)
            up = const.tile([128, 2, G], BF16)
            dw = const.tile([128, 2], BF16)
            ident = const.tile([128, 128], BF16)
            zeros = const.tile([128, 8 * BL], BF16)

            HS = state.tile([128, 2, t_steps + 1, BL], BF16)
            # cw(t): slots 0:2 = relu(g) of step t (written during sigmoid t),
            # slots 2:4 = c_{t-1} (written by step t-1's cell update).
            cw = cwp.tile([128, 4, BL], F32, tag="cw")

            nc.sync.dma_start(xt[:], xt_d[:])
            nc.sync.dma_start(wb[:], wb_d[:])
            for k in range(2):
                nc.sync.dma_start(up[:, k, :], up_d[k * 128 : (k + 1) * 128, :])
            make_identity(nc, ident[:])
            nc.vector.memset(zeros[:], 0.0)
            if chained:
                nc.sync.dma_start(HS[:, :, 0, :], hin_d[:].rearrange("p (k b) -> p k b", k=2))
                nc.sync.dma_start(cw[:, 2:4, :], cin_d[:].rearrange("p (k b) -> p k b", k=2))
            else:
                nc.vector.memset(cw[:, 2:4, :], 0.0)
                nc.vector.memset(HS[:, :, 0, :], 0.0)
            # dw is only needed by the dense head (tens of µs later) — last.
            nc.sync.dma_start(dw[:], dw_d.rearrange("(k p) one -> p (k one)", p=128))

            def w_mms(zp_t, zg_t, t):
                # start=True clears has_written for the whole PSUM bank, so
                # each group opens with ONE matmul covering its full tile
                # (sets every element's has_written); the W-matmuls and
                # U-matmuls then all accumulate with start=False. Sigmoid
                # slots and the relu(g) slots live in separate banks so
                # their readers (sigmoid on ACT, rg on DVE) don't serialize
                # on the PSUM read port.
                nc.tensor.matmul(
                    zp_t[:],
                    ident[:],
                    zeros[:, 0 : 6 * BL].rearrange("p (j b) -> p j b", j=6),
                    start=True,
                    stop=False,
                    skip_group_check=True,
                )
                nc.tensor.matmul(
                    zg_t[:],
                    ident[:],
                    zeros[:, 0 : 2 * BL].rearrange("p (j b) -> p j b", j=2),
                    start=True,
                    stop=False,
                    skip_group_check=True,
                )
                # xz injection + bias for step t, in the PE idle window
                for j in range(8):
                    dst = zp_t[:, j, :] if j < 6 else zg_t[:, j - 6, :]
                    nc.tensor.matmul(
                        dst,
                        wb[:, j * 128 : (j + 1) * 128],
                        xt[:, t * BL : (t + 1) * BL],
                        start=False,
                        stop=False,
                        skip_group_check=True,
                    )

            def head_mm(c):
                tpc = cs // BL
                sp = ppsum.tile([1, cs], F32, tag="sp")
                for k in range(2):
                    nc.tensor.matmul(
                        sp[:],
                        dw[:, k : k + 1],
                        HS[:, k, 1 + c * tpc : 1 + (c + 1) * tpc, :],
                        start=(k == 0),
                        stop=(k == 1),
                    )
                return sp

            def head_out(c, sp):
                so = tmp.tile([1, cs], F32, tag="so")
                nc.vector.tensor_copy(so[:], sp[:])
                nc.sync.dma_start(out_d[c * cs : (c + 1) * cs], so[:])

            pe_tasks = {t: [] for t in range(t_steps)}
            dve_tasks = {t: [] for t in range(t_steps)}
            for c in range(n_chunks):
                s = (c + 1) * spt - 1
                if s >= t_steps - 1:
                    continue  # emitted in the tail instead
                pe_tasks[s + 1].append(c)
                dve_tasks[s + 1].append(c)

            sps = {}

            # ---- recurrence ----
            zp = zpsum.tile([128, 6, BL], F32, tag="zp")
            zg = zgsum.tile([128, 2, BL], F32, tag="zg")
            w_mms(zp, zg, 0)
            for t in range(t_steps):
                for j in range(8):
                    dst = zp[:, j, :] if j < 6 else zg[:, j - 6, :]
                    for k in range(2):
                        nc.tensor.matmul(
                            dst,
                            up[:, k, j * 128 : (j + 1) * 128],
                            HS[:, k, t, :],
                            start=False,
                            stop=(k == 1),
                            skip_group_check=True,
                        )
                # PE idle window: next step's W-matmuls + background work
                if t + 1 < t_steps:
                    zp_next = zpsum.tile([128, 6, BL], F32, tag="zp")
                    zg_next = zgsum.tile([128, 2, BL], F32, tag="zg")
                    w_mms(zp_next, zg_next, t + 1)
                else:
                    zp_next = None
                    zg_next = None
                for c in pe_tasks.get(t, ()):
                    sps[c] = head_mm(c)

                zs = zsp.tile([128, 6, BL], F32, tag="zs")
                nc.scalar.activation(zs[:], zp[:], AF.Sigmoid)

                # relu(g) depends only on the matmuls, so it runs on DVE
                # during the sigmoid's window (own PSUM bank, no reader
                # serialization), written next to c_{t-1} in cw.
                nc.vector.tensor_scalar_max(cw[:, 0:2, :], zg[:], 0.0)
                cw_next = cwp.tile([128, 4, BL], F32, tag="cw")

                # DVE critical chain (all SBUF):
                #   t12 = [i*relu(g), f*c] in one op (zs slots are [i,f,o])
                #   c'  = t12[0:2] + t12[2:4]  -> cw_next slots 2:4
                #   h   = c' * o  (c' >= 0 always, so no relu needed)
                t12 = tmp.tile([128, 4, BL], F32, tag="t12")
                nc.vector.tensor_mul(t12[:], zs[:, 0:4, :], cw[:, 0:4, :])
                nc.vector.tensor_add(
                    cw_next[:, 2:4, :], t12[:, 0:2, :], t12[:, 2:4, :]
                )
                nc.vector.tensor_mul(
                    HS[:, :, t + 1, :], cw_next[:, 2:4, :], zs[:, 4:6, :]
                )
                cw = cw_next
                for c in dve_tasks.get(t, ()):
                    head_out(c, sps.pop(c))
                zp, zg = zp_next, zg_next

            # ---- tail ----
            # h/c state out first: ready before the head chunk's copy, so
            # their descriptor-gens don't delay the final out-DMA.
            if chained:
                nc.sync.dma_start(
                    hout_d[:].rearrange("p (k b) -> p k b", k=2),
                    HS[:, :, t_steps, :],
                )
                nc.sync.dma_start(
                    cout_d[:].rearrange("p (k b) -> p k b", k=2), cw[:, 2:4, :]
                )
            c = n_chunks - 1
            sp = head_mm(c)
            head_out(c, sp)

    nc.finalize()
    _hoist_sigmoid_waits(nc)
    return nc


def _hoist_sigmoid_waits(nc):
    """Move each recurrence sigmoid's PE wait from its SEQ-blocking
    EventSemaphore onto the sigmoid instruction itself.

    The scheduler parks the sigmoid's real dependency (PE counting-sem >=
    last U-matmul) in an EventSemaphore that blocks the ACT sequencer, so
    the sigmoid's decode+dispatch (~80ns) lands after the matmuls complete,
    on the critical path. The sigmoid's own wait slot only holds waits that
    are transitively implied: its self-ordering wait (sigma(t) complete =>
    implied because t1(t) waits sigma(t)'s completion-inc and the
    U-matmuls(t+1) wait HS(t)) and the zs-buffer WAR (readers 5 steps back).
    So: put the PE wait on the sigmoid, clear the EventSemaphore's waits
    (it stays as a cheap no-op). The sequencer then pre-decodes the sigmoid
    during the idle window and the engine fires straight off the PE inc.
    First/last two sigmoids are left untouched (startup/tail deps).
    """
    fn = nc.m.functions[0]
    act_stream = []
    for bb in fn.blocks:
        for inst in bb.instructions:
            if inst.engine == mybir.EngineType.Activation:
                act_stream.append(inst)
    sig_idx = [
        i
        for i, inst in enumerate(act_stream)
        if type(inst).__name__ == "InstActivation"
        and getattr(inst, "func", None) == AF.Sigmoid
    ]
    for i in sig_idx[2:-2]:
        sig = act_stream[i]
        prev = act_stream[i - 1]
        if type(prev).__name__ != "InstEventSemaphore":
            continue
        ev_si = prev.sync_info
        sig_si = sig.sync_info
        if ev_si is None or sig_si is None:
            continue
        pe_waits = [w for w in ev_si.on_wait if w.ant_name.startswith("PE")]
        if not pe_waits:
            continue
        keep = [w for w in sig_si.on_wait if w.ant_name.startswith("PE")]
        sig_si.on_wait = keep + pe_waits
        ev_si.on_wait = []


_PROGRAM_CACHE: dict = {}


def _get_program(t_steps: int = TSPLIT, chained: bool = True):
    key = (t_steps, chained)
    if key not in _PROGRAM_CACHE:
        _PROGRAM_CACHE[key] = build_program(t_steps, chained)
    return _PROGRAM_CACHE[key]


LAST_EXEC_TIME_NS = None


def kernel(x, W, U, b, dense_w, dense_b):
    global LAST_EXEC_TIME_NS
    x = np.asarray(x, dtype=np.float32)
    W = np.asarray(W, dtype=np.float32)
    U = np.asarray(U, dtype=np.float32)
    b = np.asarray(b, dtype=np.float32)
    dense_w = np.asarray(dense_w, dtype=np.float32)
    dense_b = np.asarray(dense_b, dtype=np.float32)

    Wp = np.ascontiguousarray(W[:, PERM])
    bp = np.ascontiguousarray(b[PERM])
    wb = np.vstack([Wp, bp[None, :]]).astype(ml_dtypes.bfloat16)
    Up = np.ascontiguousarray(U[:, PERM]).astype(ml_dtypes.bfloat16)
    dw = dense_w.astype(ml_dtypes.bfloat16)

    nc = _get_program(TSPLIT, True)
    n_parts = T // TSPLIT
    tbp = TSPLIT * BL

    h_state = [np.zeros((128, 2 * BL), ml_dtypes.bfloat16) for _ in range(NCORES)]
    c_state = [np.zeros((128, 2 * BL), np.float32) for _ in range(NCORES)]
    parts_out = []
    exec_ns = 0
    for p in range(n_parts):
        in_maps = []
        for c in range(NCORES):
            xs = x[c * BL : (c + 1) * BL, p * TSPLIT : (p + 1) * TSPLIT]
            xtc = np.empty((DB, tbp), dtype=ml_dtypes.bfloat16)
            xtc[:D] = xs.transpose(2, 1, 0).reshape(D, tbp)
            xtc[D] = 1.0
            in_maps.append(
                {
                    "xt": xtc,
                    "wb": wb,
                    "up": Up,
                    "dw": dw,
                    "hin": h_state[c],
                    "cin": c_state[c],
                }
            )
        res = run_bass_kernel_spmd(nc, in_maps, list(range(NCORES)))
        if res.exec_time_ns:
            exec_ns += res.exec_time_ns
        outs = []
        for c in range(NCORES):
            r = res.results[c]
            outs.append(np.asarray(r["out"], np.float32).reshape(TSPLIT, BL).T)
            h_state[c] = np.asarray(r["hout"])
            c_state[c] = np.asarray(r["cout"])
        parts_out.append(np.concatenate(outs, axis=0))  # [B, TSPLIT]
    LAST_EXEC_TIME_NS = exec_ns or None

    sigma = np.concatenate(parts_out, axis=1) + dense_b[0]
    return sigma.astype(np.float32)


# revision 34
# speedup vs baseline: 1.0262x; 1.0262x over previous
"""LSTM (B=64, T=512, D=64, U=256) + dense head, Trainium2 Bass kernel.

Sharding: data-parallel over batch. 8 cores x 8 sequences each, no
collectives. Everything on-device lives in "transposed" layout
[feature, batch] so gates sit on partitions and elementwise ops run with
all 128 lanes busy.

The 512-step recurrence is split into two chained 256-step programs
(a single 512-step program exceeds a per-engine instruction-count limit
on hardware); h/c state passes through DRAM between the launches.

Per recurrence step (gate columns host-permuted to [i, f, o, g]):
  PE:  two PSUM tiles in separate banks — sigmoid slots zp[128,6,BL] and
       relu-gate slots zg[128,2,BL]. Each opens with a full-tile zeros
       matmul (start=True clears has_written for the WHOLE bank, so one
       covering matmul must set every element's bit), then per slot: a
       W-matmul (wbᵀ@x_t with a ones-row folding in the bias — no xz
       precompute phase) and two U-matmuls (upᵀ@h_{t-1}, bf16) accumulate.
       Step t+1's W-matmuls are emitted after step t's U-matmuls so they
       run in the PE idle window.
  ACT: one sigmoid over zp -> zs slots [i, f, o]. A post-schedule pass
       (_hoist_sigmoid_waits) moves its PE wait out of the SEQ-blocking
       EventSemaphore so the decode happens in the idle window.
  DVE: rg = max(zg, 0) into cw[0:2] runs DURING the sigmoid (its own
       PSUM bank avoids the single-reader serialization with ACT), and
       cw[2:4] already holds c_{t-1}, so the critical chain is 3 ops:
         t12 = zs[0:4] * cw[0:4]        ([i*rg, f*c] in one op)
         c'  = t12[0:2] + t12[2:4]      (-> cw_next[2:4])
         h   = c' * o                   (c' >= 0 always, relu elided)
       The same-engine completion waits between them (~95ns) are
       hardware-required: the DVE does not forward back-to-back writes.
All pools use bufs>=3 so WAR deps land >=2 steps in the past.
"""

import numpy as np
import ml_dtypes

import concourse.bass as bass
import concourse.bacc as bacc
import concourse.mybir as mybir
import concourse.tile as tile
from concourse.bass_utils import run_bass_kernel_spmd
from concourse.masks import make_identity

B, T, D, NU = 64, 512, 64, 256
G = 4 * NU  # 1024
NCORES = 8
BL = B // NCORES  # batch per core
TSPLIT = 256  # steps per launch
DB = D + 1  # x rows + ones row (bias)

F32 = mybir.dt.float32
BF16 = mybir.dt.bfloat16
AF = mybir.ActivationFunctionType
ALU = mybir.AluOpType

# Original gate packing along the 4U axis is [i, f, g, o] (Keras order).
# On-device slot order is [i, f, o, g]: sigmoid gates contiguous in slots
# 0..5 (i,f first so t12 pairs them with [rg, c]), relu gate (g) in 6..7.
PERM = np.concatenate(
    [
        np.arange(0, 256),  # i
        np.arange(256, 512),  # f
        np.arange(768, 1024),  # o
        np.arange(512, 768),  # g
    ]
)


def build_program(t_steps: int = TSPLIT, chained: bool = True):
    tb = t_steps * BL
    nc = bacc.Bacc()

    xt_d = nc.dram_tensor("xt", [DB, tb], BF16, kind="ExternalInput")
    wb_d = nc.dram_tensor("wb", [DB, G], BF16, kind="ExternalInput")
    up_d = nc.dram_tensor("up", [NU, G], BF16, kind="ExternalInput")
    dw_d = nc.dram_tensor("dw", [NU, 1], BF16, kind="ExternalInput")
    out_d = nc.dram_tensor("out", [tb], F32, kind="ExternalOutput")
    if chained:
        hin_d = nc.dram_tensor("hin", [128, 2 * BL], BF16, kind="ExternalInput")
        cin_d = nc.dram_tensor("cin", [128, 2 * BL], BF16, kind="ExternalInput")
        hout_d = nc.dram_tensor("hout", [128, 2 * BL], BF16, kind="ExternalOutput")
        cout_d = nc.dram_tensor("cout", [128, 2 * BL], BF16, kind="ExternalOutput")

    cs = min(512, tb)  # free-dim chunk for the dense head
    n_chunks = tb // cs
    spt = cs // BL  # steps per head chunk

    with tile.TileContext(nc) as tc:
        with (
            tc.tile_pool(name="const", bufs=1) as const,
            tc.tile_pool(name="state", bufs=1) as state,
            tc.tile_pool(name="zsp", bufs=6) as zsp,
            tc.tile_pool(name="tmp", bufs=6) as tmp,
            tc.tile_pool(name="cwp", bufs=4) as cwp,
            tc.tile_pool(name="zpsum", bufs=3, space="PSUM") as zpsum,
            tc.tile_pool(name="zgsum", bufs=3, space="PSUM") as zgsum,
            tc.tile_pool(name="ppsum", bufs=2, space="PSUM") as ppsum,
        ):
            xt = const.tile([DB, tb], BF16)
            wb = const.tile([DB, G], BF16)
            up = const.tile([128, 2, G], BF16)
            dw = const.tile([128, 2], BF16)
            ident = const.tile([128, 128], BF16)
            zeros = const.tile([128, 8 * BL], BF16)

            HS = state.tile([128, 2, t_steps + 1, BL], BF16)
            # cw(t): slots 0:2 = relu(g) of step t (written during sigmoid t),
            # slots 2:4 = c_{t-1} (written by step t-1's cell update).
            cw = cwp.tile([128, 4, BL], BF16, tag="cw")

            nc.sync.dma_start(xt[:], xt_d[:])
            nc.sync.dma_start(wb[:], wb_d[:])
            for k in range(2):
                nc.sync.dma_start(up[:, k, :], up_d[k * 128 : (k + 1) * 128, :])
            make_identity(nc, ident[:])
            nc.vector.memset(zeros[:], 0.0)
            if chained:
                nc.sync.dma_start(HS[:, :, 0, :], hin_d[:].rearrange("p (k b) -> p k b", k=2))
                nc.sync.dma_start(cw[:, 2:4, :], cin_d[:].rearrange("p (k b) -> p k b", k=2))
            else:
                nc.vector.memset(cw[:, 2:4, :], 0.0)
                nc.vector.memset(HS[:, :, 0, :], 0.0)
            # dw is only needed by the dense head (tens of µs later) — last.
            nc.sync.dma_start(dw[:], dw_d.rearrange("(k p) one -> p (k one)", p=128))

            def w_mms(zp_t, zg_t, t):
                # start=True clears has_written for the whole PSUM bank, so
                # each group opens with ONE matmul covering its full tile
                # (sets every element's has_written); the W-matmuls and
                # U-matmuls then all accumulate with start=False. Sigmoid
                # slots and the relu(g) slots live in separate banks so
                # their readers (sigmoid on ACT, rg on DVE) don't serialize
                # on the PSUM read port.
                nc.tensor.matmul(
                    zp_t[:],
                    ident[:],
                    zeros[:, 0 : 6 * BL].rearrange("p (j b) -> p j b", j=6),
                    start=True,
                    stop=False,
                    skip_group_check=True,
                )
                nc.tensor.matmul(
                    zg_t[:],
                    ident[:],
                    zeros[:, 0 : 2 * BL].rearrange("p (j b) -> p j b", j=2),
                    start=True,
                    stop=False,
                    skip_group_check=True,
                )
                # xz injection + bias for step t, in the PE idle window
                for j in range(8):
                    dst = zp_t[:, j, :] if j < 6 else zg_t[:, j - 6, :]
                    nc.tensor.matmul(
                        dst,
                        wb[:, j * 128 : (j + 1) * 128],
                        xt[:, t * BL : (t + 1) * BL],
                        start=False,
                        stop=False,
                        skip_group_check=True,
                    )

            def head_mm(c):
                tpc = cs // BL
                sp = ppsum.tile([1, cs], F32, tag="sp")
                for k in range(2):
                    nc.tensor.matmul(
                        sp[:],
                        dw[:, k : k + 1],
                        HS[:, k, 1 + c * tpc : 1 + (c + 1) * tpc, :],
                        start=(k == 0),
                        stop=(k == 1),
                    )
                return sp

            def head_out(c, sp):
                so = tmp.tile([1, cs], F32, tag="so")
                nc.vector.tensor_copy(so[:], sp[:])
                nc.sync.dma_start(out_d[c * cs : (c + 1) * cs], so[:])

            pe_tasks = {t: [] for t in range(t_steps)}
            dve_tasks = {t: [] for t in range(t_steps)}
            for c in range(n_chunks):
                s = (c + 1) * spt - 1
                if s >= t_steps - 1:
                    continue  # emitted in the tail instead
                pe_tasks[s + 1].append(c)
                dve_tasks[s + 1].append(c)

            sps = {}

            # ---- recurrence ----
            zp = zpsum.tile([128, 6, BL], F32, tag="zp")
            zg = zgsum.tile([128, 2, BL], F32, tag="zg")
            w_mms(zp, zg, 0)
            for t in range(t_steps):
                for j in range(8):
                    dst = zp[:, j, :] if j < 6 else zg[:, j - 6, :]
                    for k in range(2):
                        nc.tensor.matmul(
                            dst,
                            up[:, k, j * 128 : (j + 1) * 128],
                            HS[:, k, t, :],
                            start=False,
                            stop=(k == 1),
                            skip_group_check=True,
                        )
                # PE idle window: next step's W-matmuls + background work
                if t + 1 < t_steps:
                    zp_next = zpsum.tile([128, 6, BL], F32, tag="zp")
                    zg_next = zgsum.tile([128, 2, BL], F32, tag="zg")
                    w_mms(zp_next, zg_next, t + 1)
                else:
                    zp_next = None
                    zg_next = None
                for c in pe_tasks.get(t, ()):
                    sps[c] = head_mm(c)

                zs = zsp.tile([128, 6, BL], BF16, tag="zs")
                nc.scalar.activation(zs[:], zp[:], AF.Sigmoid)

                # relu(g) depends only on the matmuls, so it runs on DVE
                # during the sigmoid's window (own PSUM bank, no reader
                # serialization), written next to c_{t-1} in cw.
                nc.vector.tensor_scalar_max(cw[:, 0:2, :], zg[:], 0.0)
                cw_next = cwp.tile([128, 4, BL], BF16, tag="cw")

                # DVE critical chain (all SBUF):
                #   t12 = [i*relu(g), f*c] in one op (zs slots are [i,f,o])
                #   c'  = t12[0:2] + t12[2:4]  -> cw_next slots 2:4
                #   h   = c' * o  (c' >= 0 always, so no relu needed)
                t12 = tmp.tile([128, 4, BL], BF16, tag="t12")
                nc.vector.tensor_mul(t12[:], zs[:, 0:4, :], cw[:, 0:4, :])
                nc.vector.tensor_add(
                    cw_next[:, 2:4, :], t12[:, 0:2, :], t12[:, 2:4, :]
                )
                nc.vector.tensor_mul(
                    HS[:, :, t + 1, :], cw_next[:, 2:4, :], zs[:, 4:6, :]
                )
                cw = cw_next
                for c in dve_tasks.get(t, ()):
                    head_out(c, sps.pop(c))
                zp, zg = zp_next, zg_next

            # ---- tail ----
            # h/c state out first: ready before the head chunk's copy, so
            # their descriptor-gens don't delay the final out-DMA.
            if chained:
                nc.sync.dma_start(
                    hout_d[:].rearrange("p (k b) -> p k b", k=2),
                    HS[:, :, t_steps, :],
                )
                nc.sync.dma_start(
                    cout_d[:].rearrange("p (k b) -> p k b", k=2), cw[:, 2:4, :]
                )
            c = n_chunks - 1
            sp = head_mm(c)
            head_out(c, sp)

    nc.finalize()
    _hoist_sigmoid_waits(nc)
    return nc


def _hoist_sigmoid_waits(nc):
    """Move each recurrence sigmoid's PE wait from its SEQ-blocking
    EventSemaphore onto the sigmoid instruction itself.

    The scheduler parks the sigmoid's real dependency (PE counting-sem >=
    last U-matmul) in an EventSemaphore that blocks the ACT sequencer, so
    the sigmoid's decode+dispatch (~80ns) lands after the matmuls complete,
    on the critical path. The sigmoid's own wait slot only holds waits that
    are transitively implied: its self-ordering wait (sigma(t) complete =>
    implied because t1(t) waits sigma(t)'s completion-inc and the
    U-matmuls(t+1) wait HS(t)) and the zs-buffer WAR (readers 5 steps back).
    So: put the PE wait on the sigmoid, clear the EventSemaphore's waits
    (it stays as a cheap no-op). The sequencer then pre-decodes the sigmoid
    during the idle window and the engine fires straight off the PE inc.
    First/last two sigmoids are left untouched (startup/tail deps).
    """
    fn = nc.m.functions[0]
    act_stream = []
    for bb in fn.blocks:
        for inst in bb.instructions:
            if inst.engine == mybir.EngineType.Activation:
                act_stream.append(inst)
    sig_idx = [
        i
        for i, inst in enumerate(act_stream)
        if type(inst).__name__ == "InstActivation"
        and getattr(inst, "func", None) == AF.Sigmoid
    ]
    for i in sig_idx[2:-2]:
        sig = act_stream[i]
        prev = act_stream[i - 1]
        if type(prev).__name__ != "InstEventSemaphore":
            continue
        ev_si = prev.sync_info
        sig_si = sig.sync_info
        if ev_si is None or sig_si is None:
            continue
        pe_waits = [w for w in ev_si.on_wait if w.ant_name.startswith("PE")]
        if not pe_waits:
            continue
        keep = [w for w in sig_si.on_wait if w.ant_name.startswith("PE")]
        sig_si.on_wait = keep + pe_waits
        ev_si.on_wait = []


_PROGRAM_CACHE: dict = {}


def _get_program(t_steps: int = TSPLIT, chained: bool = True):
    key = (t_steps, chained)
    if key not in _PROGRAM_CACHE:
        _PROGRAM_CACHE[key] = build_program(t_steps, chained)
    return _PROGRAM_CACHE[key]


LAST_EXEC_TIME_NS = None


def kernel(x, W, U, b, dense_w, dense_b):
    global LAST_EXEC_TIME_NS
    x = np.asarray(x, dtype=np.float32)
    W = np.asarray(W, dtype=np.float32)
    U = np.asarray(U, dtype=np.float32)
    b = np.asarray(b, dtype=np.float32)
    dense_w = np.asarray(dense_w, dtype=np.float32)
    dense_b = np.asarray(dense_b, dtype=np.float32)

    Wp = np.ascontiguousarray(W[:, PERM])
    bp = np.ascontiguousarray(b[PERM])
    wb = np.vstack([Wp, bp[None, :]]).astype(ml_dtypes.bfloat16)
    Up = np.ascontiguousarray(U[:, PERM]).astype(ml_dtypes.bfloat16)
    dw = dense_w.astype(ml_dtypes.bfloat16)

    nc = _get_program(TSPLIT, True)
    n_parts = T // TSPLIT
    tbp = TSPLIT * BL

    h_state = [np.zeros((128, 2 * BL), ml_dtypes.bfloat16) for _ in range(NCORES)]
    c_state = [np.zeros((128, 2 * BL), ml_dtypes.bfloat16) for _ in range(NCORES)]
    parts_out = []
    exec_ns = 0
    for p in range(n_parts):
        in_maps = []
        for c in range(NCORES):
            xs = x[c * BL : (c + 1) * BL, p * TSPLIT : (p + 1) * TSPLIT]
            xtc = np.empty((DB, tbp), dtype=ml_dtypes.bfloat16)
            xtc[:D] = xs.transpose(2, 1, 0).reshape(D, tbp)
            xtc[D] = 1.0
            in_maps.append(
                {
                    "xt": xtc,
                    "wb": wb,
                    "up": Up,
                    "dw": dw,
                    "hin": h_state[c],
                    "cin": c_state[c],
                }
            )
        res = run_bass_kernel_spmd(nc, in_maps, list(range(NCORES)))
        if res.exec_time_ns:
            exec_ns += res.exec_time_ns
        outs = []
        for c in range(NCORES):
            r = res.results[c]
            outs.append(np.asarray(r["out"], np.float32).reshape(TSPLIT, BL).T)
            h_state[c] = np.asarray(r["hout"])
            c_state[c] = np.asarray(r["cout"])
        parts_out.append(np.concatenate(outs, axis=0))  # [B, TSPLIT]
    LAST_EXEC_TIME_NS = exec_ns or None

    sigma = np.concatenate(parts_out, axis=1) + dense_b[0]
    return sigma.astype(np.float32)
